# revision 46
# baseline (speedup 1.0000x reference)
"""Multi-head self-attention (B=2, S=2048, E=1024, H=16) on 8 Trainium2 cores.

Sharding: tensor-parallel over heads -- 2 heads per core. Each core:
  - computes Q/K/V projections for its 128 E-dims (d-major layouts),
  - runs attention for its (2 heads x 2 batches),
  - emits a partial output projection (contraction over its 128 dims of Wo).
Host sums the 8 partials and adds the output bias.

All matmuls run in "transposed" space so the big P = softmax(QK^T) matrix
never needs an on-chip transpose:
  ST[k,q] = K @ Q^T        (lhsT = K^T tile, rhs = Q^T tile)
  PT      = exp(ST)        (ScalarE, straight from PSUM)
  attn^T  = V'^T P^T       (lhsT = k-major V chunk with a ones column ->
                            row 64 of the psum is the softmax row-sum)
Scale 1/sqrt(dh)=1/8 is folded into Wq/bq on the host; the V bias is applied
inside the V projection (valid because softmax rows sum to 1).

Matmul operands are bf16 except the PV stage, which runs in fp8e4m3
DoubleRow mode: the exp writes P^T as 8*exp(s) straight to fp8 pair tiles
[128, 2, 1024] (scale 8 folded into the exp bias), V' is stored fp8
pre-scaled x16, and one DoubleRow matmul contracts 256 keys (a kt pair),
halving PV tensor-engine time. Both scales cancel exactly in the
rowsum-normalization (selc carries the 16). PSUM accumulation stays fp32.
Normalization is fused into the PSUM->SBUF extract on the DVE (GPSIMD
cannot touch PSUM), using the fast custom-DVE reciprocal.

The softmax exp train on the ScalarE is the serial bottleneck (~1.05us per
[128,1024] kt); key-pair 2 of each batch is offloaded to the DVE via a
Schraudolph bit-trick exp (i32(A*s+B) viewed as f32; PV for those pairs is
bf16 off the high halves of the i32 words, with bf16 V' copies). Two
out-projection extracts per unit run on the ScalarE in the offload window
so the ACT never idles. The schedule streams projection chunks into the
attention units as deadline-placed fillers (minimal head), and each unit's
last PV pair + finish are carried into the next unit so the PE never
serializes behind the ACT at unit boundaries. Measured ~207-212us on HW
(rel err 1.75e-2), from a 216.7us bf16 baseline.
"""

import sys

sys.path.insert(0, "/opt/trn_rl_repo")

import numpy as np
import ml_dtypes

B = 2
S = 2048
E = 1024
H = 16
DH = 64
NCORES = 8
HPC = H // NCORES  # heads per core = 2
LOC = HPC * DH     # local E dims per core = 128

_CACHED = {}


def _split_waits(nc):
    """Walrus in this toolchain accepts at most ONE sync wait per instruction.
    Split any multi-wait instruction into single-wait NoOps on the same engine
    placed immediately before it (sequencer stalls are order-equivalent)."""
    import concourse.mybir as mybir

    nid = 0
    for blk in nc.m.functions[0].blocks:
        out = []
        changed = False
        for inst in blk.instructions:
            si = inst.sync_info
            if si is not None and len(si.on_wait) > 1:
                waits = list(si.on_wait)
                for w in waits[:-1]:
                    nid += 1
                    n = mybir.InstNoOp(name=f"I-waitsplit-{nid}", ins=[], outs=[])
                    n.engine = inst.engine
                    n.sync_info = mybir.SyncInfo(on_wait=[w], on_update=[])
                    out.append(n)
                inst.sync_info = mybir.SyncInfo(
                    on_wait=[waits[-1]], on_update=list(si.on_update)
                )
                changed = True
            out.append(inst)
        if changed:
            blk.instructions = out
    return nc


def build_nc(s=S, debug=False):
    """Build the per-core Bass program. `s` = sequence length (parametric so
    CoreSim checks can run on a smaller config)."""
    import concourse.bass as bass
    import concourse.mybir as mybir
    import concourse.tile as tile
    from concourse.masks import make_identity

    F32 = mybir.dt.float32
    F32R = mybir.dt.float32r
    BF16 = mybir.dt.bfloat16
    F8 = mybir.dt.float8e4
    I32 = mybir.dt.int32
    LN8 = 2.0794415416798357  # ln(8): folds pt = 8*exp(s) into the exp bias
    # Schraudolph exp on the DVE: i32(A*s + B) bit-viewed as f32 ~= 8*e^s
    # (the +3<<23 in B folds the same x8 as the ACT path's ln8 bias).
    # C=366000 tuned on the host simulation for the softmax metric.
    SCH_A = 12102203.161561485       # 2^23 / ln 2
    SCH_B = 1090153040.0             # 130*2^23 - 366000
    r = B * s              # total rows
    NCH = r // 512         # 512-wide column chunks over rows
    KT = s // 128          # 128-key tiles per batch
    QC = s // 512          # 512-wide q chunks per batch
    NTR = r // 128         # 128-row transpose tiles

    nc = bass.Bass()

    if debug:
        dbg = {
            "dbg_qt": nc.declare_dram_parameter("dbg_qt", [128, r], BF16, isOutput=True),
            "dbg_kt": nc.declare_dram_parameter("dbg_kt", [128, r], BF16, isOutput=True),
            "dbg_vp0": nc.declare_dram_parameter("dbg_vp0", [128, NTR, 2, 128], mybir.dt.float8e4, isOutput=True),
            "dbg_attn": nc.declare_dram_parameter("dbg_attn", [128, r], BF16, isOutput=True),
        }

    xT = nc.declare_dram_parameter("xT", [r // 512, 128, 8, 512], BF16, isOutput=False)
    wq = nc.declare_dram_parameter("wq", [128, 8, 128], BF16, isOutput=False)
    wk = nc.declare_dram_parameter("wk", [128, 8, 128], BF16, isOutput=False)
    wv = nc.declare_dram_parameter("wv", [128, 8, 128], BF16, isOutput=False)
    bq = nc.declare_dram_parameter("bq", [128, 1], F32, isOutput=False)
    bk = nc.declare_dram_parameter("bk", [128, 1], F32, isOutput=False)
    bv = nc.declare_dram_parameter("bv", [128, 1], F32, isOutput=False)
    wo = nc.declare_dram_parameter("wo", [128, E], BF16, isOutput=False)
    selc = nc.declare_dram_parameter("selc", [128, 128], F32R, isOutput=False)
    # partial output, partition-major: element [p, blk, c] = out row blk*128+p
    # (lets one DMA move a whole [128, 2, E] sbuf tile; host untangles)
    outp = nc.declare_dram_parameter("out", [128, r // 128, E], BF16, isOutput=True)

    with tile.TileContext(nc) as tc:
        with (
            tc.tile_pool(name="consts", bufs=1) as consts,
            tc.tile_pool(name="xt", bufs=4) as xt_pool,
            tc.tile_pool(name="qkv", bufs=1) as qkv_pool,
            tc.tile_pool(name="vtmp", bufs=2) as vtmp_pool,
            tc.tile_pool(name="pt", bufs=6) as pt_pool,
            tc.tile_pool(name="pts", bufs=2) as pts_pool,
            tc.tile_pool(name="small", bufs=4) as small_pool,
            tc.tile_pool(name="bcs", bufs=2) as bcs_pool,
            tc.tile_pool(name="osb", bufs=6) as osb_pool,
            tc.tile_pool(name="ps_mm", bufs=2, space="PSUM") as ps_mm,
            tc.tile_pool(name="ps_st", bufs=2, space="PSUM") as ps_st,
            tc.tile_pool(name="ps_pv", bufs=2, space="PSUM") as ps_pv,
        ):
            # first x chunk is on the critical path to the first matmul:
            # DMA it (in two queue-parallel halves) before the constants
            xt0 = xt_pool.tile([128, 8, 512], BF16, tag="xt", name="xt0")
            for q4, deng in enumerate((nc.sync, nc.gpsimd, nc.scalar, nc.sync)):
                deng.dma_start(
                    xt0[:, q4 * 2 : q4 * 2 + 2, :], xT[0, :, q4 * 2 : q4 * 2 + 2, :]
                )

            # ---- constants ----
            wq_sb = consts.tile([128, 8, 128], BF16, tag="wq")
            wk_sb = consts.tile([128, 8, 128], BF16, tag="wk")
            wv_sb = consts.tile([128, 8, 128], BF16, tag="wv")
            bq_sb = consts.tile([128, 1], F32, tag="bq")
            bk_sb = consts.tile([128, 1], F32, tag="bk")
            bv_sb = consts.tile([128, 1], F32, tag="bv")
            wo_sb = consts.tile([128, E], BF16, tag="wo")
            selc_sb = consts.tile([128, 128], F32R, tag="selc")
            ident = consts.tile([128, 128], BF16, tag="ident")
            ln8_sb = consts.tile([128, 1], F32, tag="ln8")
            nc.vector.memset(ln8_sb[:], LN8)
            nc.sync.dma_start(wq_sb[:], wq[:])
            nc.sync.dma_start(wk_sb[:], wk[:])
            nc.sync.dma_start(wv_sb[:], wv[:])
            nc.sync.dma_start(bq_sb[:], bq[:])
            nc.sync.dma_start(bk_sb[:], bk[:])
            nc.sync.dma_start(bv_sb[:], bv[:])
            nc.sync.dma_start(wo_sb[:], wo[:])
            nc.sync.dma_start(selc_sb[:], selc[:])
            make_identity(nc, ident[:])

            # persistent activations
            qt_sb = qkv_pool.tile([128, r], BF16, tag="qt")     # Q^T  (scaled)
            kt_sb = qkv_pool.tile([128, r], BF16, tag="kt")     # K^T
            # k-major V' for both heads: [128 keys, tile, head, 64 dims + ones]
            # fp8 (values pre-scaled x16); ones column exact in fp8.
            # (innermost padded to 128B: dual-fp8 ldweights needs the group
            # stride 128B-aligned)
            vp = qkv_pool.tile([128, NTR, 2, 128], F8, tag="vp")
            attn_sb = qkv_pool.tile([128, r], BF16, tag="attn")   # normalized attn^T
            nc.vector.memset(vp[:, :, :, 64], 1.0)
            # key-pair 0 of each batch runs Schraudolph-exp on the DVE and a
            # bf16 PV (offloads the ACT); its V' tiles live in bf16
            SCHRAUD_PAIRS = {2} if NCH == 8 else set()
            vp_bf = None
            if SCHRAUD_PAIRS:
                vp_bf = qkv_pool.tile([128, 4, 2, 65], BF16, tag="vpbf")
                nc.vector.memset(vp_bf[:, :, :, 64], 1.0)
            sch_tiles = {
                b_ * KT + 2 * p_ + g_: (b_ * len(SCHRAUD_PAIRS) + pi_) * 2 + g_
                for b_ in range(B)
                for pi_, p_ in enumerate(sorted(SCHRAUD_PAIRS))
                for g_ in range(2)
            }

            # ~5us of dummy matmuls at start: runs while the first input DMA
            # is in flight and lifts the PE HAM clock-gate to 8/8 (2.4 GHz)
            # before the real matmuls begin.
            warm_sb = consts.tile([128, 512], BF16, tag="warm")
            nc.vector.memset(warm_sb[:], 0.0)
            warm_ps = ps_mm.tile([128, 512], F32, tag="mm", name="warmps")
            NWARM = 10
            for wi in range(NWARM):
                nc.tensor.matmul(
                    warm_ps[:],
                    warm_sb[:, 0:128],
                    warm_sb[:],
                    start=(wi == 0),
                    stop=(wi == NWARM - 1),
                )

            # ---- phase A: projections (d-major) + V transpose to k-major ----
            # V transposes are deferred by one chunk so the PE never stalls
            # on the freshly-written vtmp (its bias-copy is one proj-group
            # old by the time the transposes dispatch).
            def emit_transposes(nch_v, vtmp_v):
                for t4 in range(4):
                    trp = ps_mm.tile([128, 2, 64], BF16, tag="mm", name="trp")
                    nc.tensor.transpose(
                        trp[:, :, :], vtmp_v[:, t4 * 128 : (t4 + 1) * 128], ident[:]
                    )
                    tg = nch_v * 4 + t4
                    # one strided copy: both heads' 64 dims, skipping the
                    # ones column at free offset 64 of each head block
                    if tg in sch_tiles:
                        nc.vector.tensor_copy(
                            vp_bf[:, sch_tiles[tg], :, 0:64], trp[:, :, :]
                        )
                    else:
                        nc.vector.tensor_copy(vp[:, tg, :, 0:64], trp[:, :, :])

            pending_tr = [None]

            def emit_proj_chunk(nch):
                if nch == 0:
                    xt = xt0
                else:
                    xt = xt_pool.tile([128, 8, 512], BF16, tag="xt", name="xt")
                    nc.sync.dma_start(xt[:, 0:4, :], xT[nch, :, 0:4, :])
                    nc.sync.dma_start(xt[:, 4:8, :], xT[nch, :, 4:8, :])
                c0 = nch * 512
                for w_sb, b_sb, dest in (
                    (wq_sb, bq_sb, qt_sb),
                    (wk_sb, bk_sb, kt_sb),
                    (wv_sb, bv_sb, None),
                ):
                    ps = ps_mm.tile([128, 512], F32, tag="mm", name="projps")
                    for kc in range(8):
                        nc.tensor.matmul(
                            ps[:],
                            w_sb[:, kc, :],
                            xt[:, kc, :],
                            start=(kc == 0),
                            stop=(kc == 7),
                        )
                    if dest is not None:
                        nc.vector.tensor_scalar_add(
                            dest[:, c0 : c0 + 512], ps[:], b_sb[:, 0:1]
                        )
                    else:
                        vtmp = vtmp_pool.tile([128, 512], BF16, tag="vtmp")
                        nc.vector.tensor_scalar(
                            vtmp[:], ps[:], b_sb[:, 0:1], 16.0,
                            mybir.AluOpType.add, mybir.AluOpType.mult,
                        )
                        if pending_tr[0] is not None:
                            emit_transposes(*pending_tr[0])
                        pending_tr[0] = (nch, vtmp)

            def proj_chunk_fillers(nch, xt_preset=None):
                state = {}
                if xt_preset is not None:
                    state["xt"] = xt_preset
                qa, qb = (nc.sync, nc.gpsimd) if nch % 2 else (nc.gpsimd, nc.sync)

                def load():
                    if "xt" in state:
                        return
                    xt = xt_pool.tile([128, 8, 512], BF16, tag="xt", name="xt")
                    qa.dma_start(xt[:, 0:4, :], xT[nch, :, 0:4, :])
                    qb.dma_start(xt[:, 4:8, :], xT[nch, :, 4:8, :])
                    state["xt"] = xt

                def group(w_sb, b_sb, dest):
                    # two 4-matmul halves so a filler burst never exceeds
                    # ~1us of PE time (keeps the exp train fed)
                    gstate = {}

                    def _a():
                        load()
                        xt = state["xt"]
                        ps = ps_mm.tile([128, 512], F32, tag="mm", name="projps")
                        gstate["ps"] = ps
                        for kc in range(4):
                            nc.tensor.matmul(
                                ps[:], w_sb[:, kc, :], xt[:, kc, :],
                                start=(kc == 0), stop=False,
                            )

                    def _b():
                        xt = state["xt"]
                        ps = gstate["ps"]
                        c0 = nch * 512
                        for kc in range(4, 8):
                            nc.tensor.matmul(
                                ps[:], w_sb[:, kc, :], xt[:, kc, :],
                                start=False, stop=(kc == 7),
                            )
                        if dest is not None:
                            nc.vector.tensor_scalar_add(
                                dest[:, c0 : c0 + 512], ps[:], b_sb[:, 0:1]
                            )
                        else:
                            vtmp = vtmp_pool.tile([128, 512], BF16, tag="vtmp")
                            nc.vector.tensor_scalar(
                                vtmp[:], ps[:], b_sb[:, 0:1], 16.0,
                                mybir.AluOpType.add, mybir.AluOpType.mult,
                            )
                            if pending_tr[0] is not None:
                                emit_transposes(*pending_tr[0])
                            pending_tr[0] = (nch, vtmp)
                    return _a, _b

                qa_, qb_ = group(wq_sb, bq_sb, qt_sb)
                ka_, kb_ = group(wk_sb, bk_sb, kt_sb)
                va_, vb_ = group(wv_sb, bv_sb, None)
                return {
                    "load": load,
                    "Qa": qa_, "Qb": qb_,
                    "Ka": ka_, "Kb": kb_,
                    "Va": va_, "Vb": vb_,
                    "Q": lambda: (qa_(), qb_()),
                    "K": lambda: (ka_(), kb_()),
                    "V": lambda: (va_(), vb_()),
                }

            def flush_tr():
                if pending_tr[0] is not None:
                    emit_transposes(*pending_tr[0])
                    pending_tr[0] = None

            # ---- phase B: attention units, software-pipelined ----
            # Per unit: 16-kt loop of (ST pair -> exp into fp8 pair tile ->
            # DoubleRow PV per kt-pair, lagged PV_LAG pairs). The previous
            # unit's finish (rowsum bcast + reciprocal + fused extract*norm)
            # runs at kt 1 of the current unit, and the unit-before-that's
            # output projection is spread over kt 8..15, so the PE never
            # drains at unit boundaries.
            PV_LAG = 2  # in kt-pairs

            # pre-zero the rowsum staging buffers (all 4 pool slots) so the
            # broadcast matmul's unused contraction rows 1..31 contribute
            # 0 * 0 instead of 0 * garbage
            for _ in range(4):
                _rs = small_pool.tile([33, 512], F32R, tag="rs")
                nc.vector.memset(_rs[:].bitcast(F32), 0.0)

            def emit_unit_kt(b, qc, slot_fns=None):
                slot_fns = dict(slot_fns or {})
                gq = b * s + qc * 512
                pvp0 = ps_pv.tile([65, 512], F32, tag="pv", name="pvp0")
                pvp1 = ps_pv.tile([65, 512], F32, tag="pv", name="pvp1")
                pv_tiles = [pvp0, pvp1]
                NP = KT // 2  # kt-pairs per unit

                def emit_pv(pair_v, pt_v):
                    if pair_v in SCHRAUD_PAIRS:
                        # bf16 PV off the high halves of the i32 Schraudolph
                        # words (bf16 = top 16 bits of f32)
                        ptv = pt_v[:].bitcast(BF16)  # [128, 2, 2048]
                        for h in range(2):
                            for g in range(2):
                                nc.tensor.matmul(
                                    pv_tiles[h][:],
                                    vp_bf[:, sch_tiles[b * KT + 2 * pair_v + g], h, 0:65],
                                    ptv[:, g, 1024 * h + 1 : 1024 * (h + 1) : 2],
                                    start=(pair_v == 0 and g == 0),
                                    stop=(pair_v == NP - 1 and g == 1),
                                )
                        return
                    # fp8 DoubleRow: one matmul covers both kt of the pair
                    # (contraction 256 keys); lhsT [128, 2, 65], rhs [128, 2, 512]
                    for h in range(2):
                        nc.tensor.matmul(
                            pv_tiles[h][:],
                            vp[:, b * KT + 2 * pair_v : b * KT + 2 * pair_v + 2, h, 0:65],
                            pt_v[:, :, h * 512 : h * 512 + 512],
                            start=(pair_v == 0),
                            stop=(pair_v == NP - 1),
                            perf_mode=mybir.MatmulPerfMode.DoubleRow,
                        )

                pending_pv = []
                pt_pair = None
                for kt in range(KT):
                    kcol = b * s + kt * 128
                    stp = ps_st.tile([128, 1024], F32, tag="st")
                    for h in range(2):
                        p0 = h * 64
                        nc.tensor.matmul(
                            stp[:, h * 512 : h * 512 + 512],
                            kt_sb[p0 : p0 + 64, kcol : kcol + 128],
                            qt_sb[p0 : p0 + 64, gq : gq + 512],
                            start=True,
                            stop=True,
                        )
                    sch = (kt // 2) in SCHRAUD_PAIRS
                    if kt % 2 == 0:
                        if sch:
                            pt_pair = pts_pool.tile(
                                [128, 2, 1024], I32, tag="pts", name="pts"
                            )
                        else:
                            pt_pair = pt_pool.tile(
                                [128, 2, 1024], F8, tag="pt", name="ptp"
                            )
                    if sch:
                        nc.vector.tensor_scalar(
                            pt_pair[:, kt % 2, :], stp[:], SCH_A, SCH_B,
                            mybir.AluOpType.mult, mybir.AluOpType.add,
                        )
                    else:
                        # pt = 8*exp(s): the 8 folds into the exp bias and
                        # cancels against the rowsum in normalization
                        nc.scalar.activation(
                            pt_pair[:, kt % 2, :], stp[:],
                            mybir.ActivationFunctionType.Exp, bias=ln8_sb[:, 0:1],
                        )
                    if kt % 2 == 1:
                        pending_pv.append((kt // 2, pt_pair))
                        if len(pending_pv) > PV_LAG - 1:
                            emit_pv(*pending_pv.pop(0))
                    for f in slot_fns.pop(kt, ()):
                        f()
                for kt in sorted(slot_fns):
                    for f in slot_fns.pop(kt):
                        f()
                # remaining PV pairs are flushed lazily by the caller (into
                # the next unit's first slots, after its first STs issue)
                return {"b": b, "qc": qc, "gq": gq, "pv": pv_tiles,
                        "emit_pv": emit_pv, "pending": pending_pv}

            def emit_finish_a(u):
                # rowsums (psum row 64 of each head) -> partitions 0/32 of one
                # sbuf tile -> single PE broadcast (contraction rows 1..31 are
                # zero in selc AND pre-zeroed in rs2) -> fast reciprocal
                rs2 = small_pool.tile([33, 512], F32R, tag="rs")
                nc.vector.tensor_copy(rs2[0:1, :], u["pv"][0][64:65, :])
                nc.vector.tensor_copy(rs2[32:33, :], u["pv"][1][64:65, :])
                # bcp lives in the mm pool so the ST psum ring stays a pure
                # (ST matmul -> exp) pipeline
                bcp = ps_mm.tile([128, 512], F32, tag="mm", name="bcp")
                nc.tensor.matmul(
                    bcp[:],
                    selc_sb[0:33, :],
                    rs2[0:33, :],
                    start=True,
                    stop=True,
                )
                bcs = bcs_pool.tile([128, 512], F32, tag="bcs")
                nc.vector.reciprocal_approx_fast(bcs[:], bcp[:])
                u["bcs"] = bcs

            def emit_finish_b(u):
                # extract attn bands from PSUM with the normalization fused in
                gq = u["gq"]
                bcs = u["bcs"]
                for h in range(2):
                    p0 = h * 64
                    nc.vector.tensor_tensor(
                        attn_sb[p0 : p0 + 64, gq : gq + 512],
                        u["pv"][h][0:64, :],
                        bcs[p0 : p0 + 64, :],
                        mybir.AluOpType.mult,
                    )

            def emit_unit_finish(u):
                emit_finish_a(u)
                emit_finish_b(u)

            def out_pieces(u, use_scalar=False, act_set=()):
                # 8 outproj pieces, each split into a matmul part and an
                # extract part so the extracts can land on the ACT inside
                # the Schraudolph window without their matmuls delaying STs
                gq = u["gq"]
                blk0 = gq // 128
                state = {}
                mms, exts = [], []
                for qb in range(4):
                    for no2 in range(2):
                        def _mm(qb=qb, no2=no2):
                            col = gq + qb * 128
                            ops = ps_mm.tile([128, 512], F32, tag="mm", name="ops")
                            state[(qb, no2)] = ops
                            nc.tensor.matmul(
                                ops[:],
                                attn_sb[:, col : col + 128],
                                wo_sb[:, no2 * 512 : (no2 + 1) * 512],
                                start=True,
                                stop=True,
                            )

                        def _ext(qb=qb, no2=no2):
                            qbp, qbs = qb // 2, qb % 2
                            if qbs == 0 and no2 == 0:
                                state[("osb", qbp)] = osb_pool.tile(
                                    [128, 2, E], BF16, tag="osb", name="osb"
                                )
                            osb = state[("osb", qbp)]
                            ops = state.pop((qb, no2))
                            dst = osb[:, qbs, no2 * 512 : (no2 + 1) * 512]
                            if (use_scalar and no2 == 1) or (qb * 2 + no2) in act_set:
                                nc.scalar.activation(
                                    dst, ops[:],
                                    mybir.ActivationFunctionType.Copy,
                                )
                            else:
                                nc.vector.tensor_copy(dst, ops[:])
                            if qbs == 1 and no2 == 1:
                                nc.sync.dma_start(
                                    outp[:, blk0 + 2 * qbp : blk0 + 2 * qbp + 2, :],
                                    osb[:, :, :],
                                )
                        mms.append(_mm)
                        exts.append(_ext)
                return mms, exts

            # Emission schedule (v2): minimal head -- chunk 0 (Q,K,V) and
            # chunk 1 (K,V) -- so the exp train starts as soon as chunk 0
            # lands. Everything else rides inside the units as deadline-
            # placed fillers; leftover PV pairs of each unit flush lazily at
            # the start of the next unit (after its first STs issue) so the
            # PE never serializes behind the ACT at unit boundaries.
            units = [(b_, qc_) for b_ in range(B) for qc_ in range(QC)]
            if NCH == 8:
                fill = {c: proj_chunk_fillers(c) for c in range(1, NCH)}
                fill[0] = proj_chunk_fillers(0, xt_preset=xt0)
                for c in (1, 2, 3):
                    fill[c]["load"]()
                fill[0]["Q"]()
                fill[0]["K"]()
                fill[0]["V"]()
                fill[1]["K"]()
                fill[1]["V"]()
                # unit-idx -> [(slot, kind, chunk)] with deadlines:
                #   K c by kt 4c of its batch-units; V c transposes by the
                #   PV pair that reads its vp tiles; Q c before unit c starts
                SLOTS = {
                    0: [(1, "Ka", 2), (2, "load", 4), (3, "Kb", 2),
                        (4, "Va", 2), (5, "Vb", 2), (6, "flush", 0),
                        (7, "Ka", 3), (8, "Va", 3), (9, "Kb", 3),
                        (10, "Vb", 3), (12, "flush", 0), (14, "Qa", 1),
                        (15, "Qb", 1)],
                    1: [(0, "load", 5), (7, "Ka", 4), (9, "Kb", 4),
                        (11, "Va", 4), (13, "Vb", 4), (14, "Qa", 2),
                        (15, "Qb", 2)],
                    2: [(0, "load", 6), (7, "Ka", 5), (9, "Kb", 5),
                        (11, "Va", 5), (13, "Vb", 5), (14, "Qa", 3),
                        (15, "Qb", 3)],
                    3: [(0, "load", 7), (7, "Ka", 6), (9, "Kb", 6),
                        (11, "Va", 6), (13, "Vb", 6), (14, "Qa", 4),
                        (15, "Qb", 4)],
                    4: [(3, "Ka", 7), (5, "Kb", 7), (7, "Va", 7),
                        (9, "Vb", 7), (11, "flush", 0), (14, "Qa", 5),
                        (15, "Qb", 5)],
                    5: [(14, "Qa", 6), (15, "Qb", 6)],
                    6: [(14, "Qa", 7), (15, "Qb", 7)],
                    7: [],
                }
            else:
                for nch in range(NCH):
                    emit_proj_chunk(nch)
                flush_tr()
                SLOTS = {i: [] for i in range(len(units))}
            prev = None
            carry_pv = None
            for idx, (b_, qc_) in enumerate(units):
                slots = {}
                if carry_pv is not None:
                    fn, pairs = carry_pv
                    for i, args in enumerate(pairs):
                        slots.setdefault(2 * i, []).append(
                            lambda a=args, f=fn: f(*a)
                        )
                if prev is not None:
                    # both finish halves must be emitted before this unit's
                    # first PV pop (kt3): the pv psum ring WAR only covers
                    # readers emitted before the ring's next first-write
                    slots.setdefault(1, []).append(
                        lambda u=prev: emit_finish_a(u)
                    )
                    slots.setdefault(2, []).append(
                        lambda u=prev: emit_finish_b(u)
                    )
                    mms, exts = out_pieces(prev, act_set=(0, 1))
                    for i in range(8):
                        if i < 2:
                            slots.setdefault(2 + i, []).append(mms[i])
                            slots.setdefault(4 + i, []).append(exts[i])
                        else:
                            slots.setdefault(4 + i, []).append(mms[i])
                            slots.setdefault(4 + i, []).append(exts[i])
                for slot, kind, arg in SLOTS.get(idx, ()):
                    if kind == "load":
                        slots.setdefault(slot, []).append(fill[arg]["load"])
                    elif kind == "flush":
                        slots.setdefault(slot, []).append(flush_tr)
                    else:
                        slots.setdefault(slot, []).append(fill[arg][kind])
                u = emit_unit_kt(b_, qc_, slots)
                prev = u
                carry_pv = (u["emit_pv"], u["pending"])
            fn, pairs = carry_pv
            for args in pairs:
                fn(*args)
            emit_finish_a(prev)
            emit_finish_b(prev)
            mms, exts = out_pieces(prev, use_scalar=True)
            for m, e in zip(mms, exts):
                m()
                e()
            if debug:
                nc.sync.dma_start(dbg["dbg_qt"][:], qt_sb[:])
                nc.sync.dma_start(dbg["dbg_kt"][:], kt_sb[:])
                nc.sync.dma_start(dbg["dbg_vp0"][:], vp[:])
                nc.sync.dma_start(dbg["dbg_attn"][:], attn_sb[:])
    return nc


def _prep_inputs(inputs, Wq, bq, Wk, bk, Wv, bv, Wo, bo, s=S):
    """Host-side shard + relayout. Returns per-core input maps."""
    inputs = np.asarray(inputs, dtype=np.float32)
    Wq, Wk, Wv, Wo = (np.asarray(w, dtype=np.float32) for w in (Wq, Wk, Wv, Wo))
    bq, bk, bv = (np.asarray(b_, dtype=np.float32) for b_ in (bq, bk, bv))
    r = B * s
    x = np.ascontiguousarray(inputs, dtype=np.float32).reshape(r, E)
    # [E, r] -> per-512-chunk contiguous tiles [NCH, 128(part), 8(kc), 512]
    xT = np.ascontiguousarray(
        x.T.reshape(8, 128, r // 512, 512).transpose(2, 1, 0, 3)
    ).astype(ml_dtypes.bfloat16)

    # 16.0: folds the 1/16 v-prescale into the reciprocal of the rowsum
    selc = np.zeros((128, 128), dtype=np.float32)
    selc[0, 0:64] = 16.0
    selc[32, 64:128] = 16.0

    def wslice(W, c):
        # W[c*128:(c+1)*128, :] transposed -> [E, 128] -> [128(part), 8, 128]
        wt = np.ascontiguousarray(
            W[c * 128 : (c + 1) * 128, :].T.reshape(8, 128, 128).transpose(1, 0, 2)
        )
        return wt

    in_maps = []
    for c in range(NCORES):
        m = {
            "xT": xT,
            "wq": (wslice(Wq, c) * 0.125).astype(ml_dtypes.bfloat16),
            "wk": wslice(Wk, c).astype(ml_dtypes.bfloat16),
            "wv": wslice(Wv, c).astype(ml_dtypes.bfloat16),
            "bq": (bq[c * 128 : (c + 1) * 128] * 0.125).reshape(128, 1).astype(np.float32),
            "bk": bk[c * 128 : (c + 1) * 128].reshape(128, 1).astype(np.float32),
            "bv": bv[c * 128 : (c + 1) * 128].reshape(128, 1).astype(np.float32),
            "wo": np.ascontiguousarray(Wo[:, c * 128 : (c + 1) * 128].T).astype(
                ml_dtypes.bfloat16
            ),
            "selc": selc,
        }
        in_maps.append(m)
    return in_maps


def _get_nc(s=S):
    if s not in _CACHED:
        import concourse.mybir as mybir

        nc = _split_waits(build_nc(s))
        # populate .instr bytes for extended InstISA subclasses (the custom
        # DVE reciprocal) -- raw Bass skips this pass and the NEFF compiler
        # errors with "ISA wrong length" without it
        mybir.codegen_inst_isa_subclasses(nc)
        _CACHED[s] = nc
    return _CACHED[s]


def kernel(
    inputs, Wq, bq, Wk, bk, Wv, bv, Wo, bo, _trace=False, _result_box=None
):
    from concourse.bass_utils import run_bass_kernel_spmd

    nc = _get_nc(S)
    in_maps = _prep_inputs(inputs, Wq, bq, Wk, bk, Wv, bv, Wo, bo)
    res = run_bass_kernel_spmd(nc, in_maps, list(range(NCORES)), trace=_trace)
    if _result_box is not None:
        _result_box.append(res)
    acc = np.zeros((B * S, E), dtype=np.float32)
    for rmap in res.results:
        # device layout [128, r//128, E]: element [p, blk, c] = row blk*128+p
        part = np.asarray(rmap["out"], dtype=np.float32)
        acc += part.transpose(1, 0, 2).reshape(B * S, E)
    acc += np.asarray(bo, dtype=np.float32)[None, :]
    return acc.reshape(B, S, E)



# revision 47
# speedup vs baseline: 1.0398x; 1.0398x over previous
"""Multi-head self-attention (B=2, S=2048, E=1024, H=16) on 8 Trainium2 cores.

Sharding: tensor-parallel over heads -- 2 heads per core. Each core:
  - computes Q/K/V projections for its 128 E-dims (d-major layouts),
  - runs attention for its (2 heads x 2 batches),
  - emits a partial output projection (contraction over its 128 dims of Wo).
Host sums the 8 partials and adds the output bias.

All matmuls run in "transposed" space so the big P = softmax(QK^T) matrix
never needs an on-chip transpose:
  ST[k,q] = K @ Q^T        (lhsT = K^T tile, rhs = Q^T tile)
  PT      = exp(ST)        (ScalarE, straight from PSUM)
  attn^T  = V'^T P^T       (lhsT = k-major V chunk with a ones column ->
                            row 64 of the psum is the softmax row-sum)
Scale 1/sqrt(dh)=1/8 is folded into Wq/bq on the host; the V bias is applied
inside the V projection (valid because softmax rows sum to 1).

Matmul operands are bf16 except the PV stage, which runs in fp8e4m3
DoubleRow mode: the exp writes P^T as 8*exp(s) straight to fp8 pair tiles
[128, 2, 1024] (scale 8 folded into the exp bias), V' is stored fp8
pre-scaled x16, and one DoubleRow matmul contracts 256 keys (a kt pair),
halving PV tensor-engine time. Both scales cancel exactly in the
rowsum-normalization (selc carries the 16). PSUM accumulation stays fp32.
Normalization is fused into the PSUM->SBUF extract on the DVE (GPSIMD
cannot touch PSUM), using the fast custom-DVE reciprocal.

The softmax exp train on the ScalarE is the serial bottleneck (~1.05us per
[128,1024] kt); key-pair 2 of each batch is offloaded to the DVE via a
Schraudolph bit-trick exp (i32(A*s+B) viewed as f32; PV for those pairs is
bf16 off the high halves of the i32 words, with bf16 V' copies). Two
out-projection extracts per unit run on the ScalarE in the offload window
so the ACT never idles. The schedule streams projection chunks into the
attention units as deadline-placed fillers (minimal head), and each unit's
last PV pair + finish are carried into the next unit so the PE never
serializes behind the ACT at unit boundaries. Measured ~207-212us on HW
(rel err 1.75e-2), from a 216.7us bf16 baseline.
"""

import sys

sys.path.insert(0, "/opt/trn_rl_repo")

import numpy as np
import ml_dtypes

B = 2
S = 2048
E = 1024
H = 16
DH = 64
NCORES = 8
HPC = H // NCORES  # heads per core = 2
LOC = HPC * DH     # local E dims per core = 128

_CACHED = {}


def _split_waits(nc):
    """Walrus in this toolchain accepts at most ONE sync wait per instruction.
    Split any multi-wait instruction into single-wait NoOps on the same engine
    placed immediately before it (sequencer stalls are order-equivalent)."""
    import concourse.mybir as mybir

    nid = 0
    for blk in nc.m.functions[0].blocks:
        out = []
        changed = False
        for inst in blk.instructions:
            si = inst.sync_info
            if si is not None and len(si.on_wait) > 1:
                waits = list(si.on_wait)
                for w in waits[:-1]:
                    nid += 1
                    n = mybir.InstNoOp(name=f"I-waitsplit-{nid}", ins=[], outs=[])
                    n.engine = inst.engine
                    n.sync_info = mybir.SyncInfo(on_wait=[w], on_update=[])
                    out.append(n)
                inst.sync_info = mybir.SyncInfo(
                    on_wait=[waits[-1]], on_update=list(si.on_update)
                )
                changed = True
            out.append(inst)
        if changed:
            blk.instructions = out
    return nc


def build_nc(s=S, debug=False):
    """Build the per-core Bass program. `s` = sequence length (parametric so
    CoreSim checks can run on a smaller config)."""
    import concourse.bass as bass
    import concourse.mybir as mybir
    import concourse.tile as tile
    from concourse.masks import make_identity

    F32 = mybir.dt.float32
    F32R = mybir.dt.float32r
    BF16 = mybir.dt.bfloat16
    F8 = mybir.dt.float8e4
    I32 = mybir.dt.int32
    LN8 = 2.0794415416798357  # ln(8): folds pt = 8*exp(s) into the exp bias
    # Schraudolph exp on the DVE: i32(A*s + B) bit-viewed as f32 ~= 8*e^s
    # (the +3<<23 in B folds the same x8 as the ACT path's ln8 bias).
    # C=366000 tuned on the host simulation for the softmax metric.
    SCH_A = 12102203.161561485       # 2^23 / ln 2
    SCH_B = 1090153040.0             # 130*2^23 - 366000
    r = B * s              # total rows
    NCH = r // 512         # 512-wide column chunks over rows
    KT = s // 128          # 128-key tiles per batch
    QC = s // 512          # 512-wide q chunks per batch
    NTR = r // 128         # 128-row transpose tiles

    nc = bass.Bass()

    if debug:
        dbg = {
            "dbg_qt": nc.declare_dram_parameter("dbg_qt", [128, r], BF16, isOutput=True),
            "dbg_kt": nc.declare_dram_parameter("dbg_kt", [128, r], BF16, isOutput=True),
            "dbg_vp0": nc.declare_dram_parameter("dbg_vp0", [128, NTR, 2, 128], mybir.dt.float8e4, isOutput=True),
            "dbg_attn": nc.declare_dram_parameter("dbg_attn", [128, r], BF16, isOutput=True),
        }

    xT = nc.declare_dram_parameter("xT", [r // 512, 128, 8, 512], BF16, isOutput=False)
    wq = nc.declare_dram_parameter("wq", [128, 8, 128], BF16, isOutput=False)
    wk = nc.declare_dram_parameter("wk", [128, 8, 128], BF16, isOutput=False)
    wv = nc.declare_dram_parameter("wv", [128, 8, 128], BF16, isOutput=False)
    bq = nc.declare_dram_parameter("bq", [128, 1], F32, isOutput=False)
    bk = nc.declare_dram_parameter("bk", [128, 1], F32, isOutput=False)
    bv = nc.declare_dram_parameter("bv", [128, 1], F32, isOutput=False)
    wo = nc.declare_dram_parameter("wo", [128, E], BF16, isOutput=False)
    selc = nc.declare_dram_parameter("selc", [128, 128], F32R, isOutput=False)
    # partial output, partition-major: element [p, blk, c] = out row blk*128+p
    # (lets one DMA move a whole [128, 2, E] sbuf tile; host untangles)
    outp = nc.declare_dram_parameter("out", [128, r // 128, E], BF16, isOutput=True)

    with tile.TileContext(nc) as tc:
        with (
            tc.tile_pool(name="consts", bufs=1) as consts,
            tc.tile_pool(name="xt", bufs=4) as xt_pool,
            tc.tile_pool(name="qkv", bufs=1) as qkv_pool,
            tc.tile_pool(name="vtmp", bufs=2) as vtmp_pool,
            tc.tile_pool(name="pt", bufs=6) as pt_pool,
            tc.tile_pool(name="pts", bufs=2) as pts_pool,
            tc.tile_pool(name="small", bufs=4) as small_pool,
            tc.tile_pool(name="bcs", bufs=2) as bcs_pool,
            tc.tile_pool(name="osb", bufs=6) as osb_pool,
            tc.tile_pool(name="ps_mm", bufs=2, space="PSUM") as ps_mm,
            tc.tile_pool(name="ps_st", bufs=2, space="PSUM") as ps_st,
            tc.tile_pool(name="ps_pv", bufs=2, space="PSUM") as ps_pv,
        ):
            # first x chunk is on the critical path to the first matmul:
            # DMA it (in two queue-parallel halves) before the constants
            xt0 = xt_pool.tile([128, 8, 512], BF16, tag="xt", name="xt0")
            for q4, deng in enumerate((nc.sync, nc.gpsimd, nc.scalar, nc.sync)):
                deng.dma_start(
                    xt0[:, q4 * 2 : q4 * 2 + 2, :], xT[0, :, q4 * 2 : q4 * 2 + 2, :]
                )

            # ---- constants ----
            wq_sb = consts.tile([128, 8, 128], BF16, tag="wq")
            wk_sb = consts.tile([128, 8, 128], BF16, tag="wk")
            wv_sb = consts.tile([128, 8, 128], BF16, tag="wv")
            bq_sb = consts.tile([128, 1], F32, tag="bq")
            bk_sb = consts.tile([128, 1], F32, tag="bk")
            bv_sb = consts.tile([128, 1], F32, tag="bv")
            wo_sb = consts.tile([128, E], BF16, tag="wo")
            selc_sb = consts.tile([128, 128], F32R, tag="selc")
            ident = consts.tile([128, 128], BF16, tag="ident")
            ln8_sb = consts.tile([128, 1], F32, tag="ln8")
            nc.vector.memset(ln8_sb[:], LN8)
            nc.sync.dma_start(wq_sb[:], wq[:])
            nc.sync.dma_start(wk_sb[:], wk[:])
            nc.sync.dma_start(wv_sb[:], wv[:])
            nc.sync.dma_start(bq_sb[:], bq[:])
            nc.sync.dma_start(bk_sb[:], bk[:])
            nc.sync.dma_start(bv_sb[:], bv[:])
            nc.sync.dma_start(wo_sb[:], wo[:])
            nc.sync.dma_start(selc_sb[:], selc[:])
            make_identity(nc, ident[:])

            # persistent activations
            qt_sb = qkv_pool.tile([128, r], BF16, tag="qt")     # Q^T  (scaled)
            kt_sb = qkv_pool.tile([128, r], BF16, tag="kt")     # K^T
            # k-major V' for both heads: [128 keys, tile, head, 64 dims + ones]
            # fp8 (values pre-scaled x16); ones column exact in fp8.
            # (innermost padded to 128B: dual-fp8 ldweights needs the group
            # stride 128B-aligned)
            vp = qkv_pool.tile([128, NTR, 2, 128], F8, tag="vp")
            attn_sb = qkv_pool.tile([128, r], BF16, tag="attn")   # normalized attn^T
            nc.vector.memset(vp[:, :, :, 64], 1.0)
            # key-pair 0 of each batch runs Schraudolph-exp on the DVE and a
            # bf16 PV (offloads the ACT); its V' tiles live in bf16
            SCHRAUD_PAIRS = {2} if NCH == 8 else set()
            vp_bf = None
            if SCHRAUD_PAIRS:
                vp_bf = qkv_pool.tile([128, 4, 2, 65], BF16, tag="vpbf")
                nc.vector.memset(vp_bf[:, :, :, 64], 1.0)
            sch_tiles = {
                b_ * KT + 2 * p_ + g_: (b_ * len(SCHRAUD_PAIRS) + pi_) * 2 + g_
                for b_ in range(B)
                for pi_, p_ in enumerate(sorted(SCHRAUD_PAIRS))
                for g_ in range(2)
            }

            # ~5us of dummy matmuls at start: runs while the first input DMA
            # is in flight and lifts the PE HAM clock-gate to 8/8 (2.4 GHz)
            # before the real matmuls begin.
            warm_sb = consts.tile([128, 512], BF16, tag="warm")
            nc.vector.memset(warm_sb[:], 0.0)
            warm_ps = ps_mm.tile([128, 512], F32, tag="mm", name="warmps")
            NWARM = 10
            for wi in range(NWARM):
                nc.tensor.matmul(
                    warm_ps[:],
                    warm_sb[:, 0:128],
                    warm_sb[:],
                    start=(wi == 0),
                    stop=(wi == NWARM - 1),
                )

            # ---- phase A: projections (d-major) + V transpose to k-major ----
            # V transposes are deferred by one chunk so the PE never stalls
            # on the freshly-written vtmp (its bias-copy is one proj-group
            # old by the time the transposes dispatch).
            def emit_transposes(nch_v, vtmp_v):
                for t4 in range(4):
                    trp = ps_mm.tile([128, 2, 64], BF16, tag="mm", name="trp")
                    nc.tensor.transpose(
                        trp[:, :, :], vtmp_v[:, t4 * 128 : (t4 + 1) * 128], ident[:]
                    )
                    tg = nch_v * 4 + t4
                    # one strided copy: both heads' 64 dims, skipping the
                    # ones column at free offset 64 of each head block
                    if tg in sch_tiles:
                        nc.vector.tensor_copy(
                            vp_bf[:, sch_tiles[tg], :, 0:64], trp[:, :, :]
                        )
                    else:
                        nc.vector.tensor_copy(vp[:, tg, :, 0:64], trp[:, :, :])

            pending_tr = [None]

            def emit_proj_chunk(nch):
                if nch == 0:
                    xt = xt0
                else:
                    xt = xt_pool.tile([128, 8, 512], BF16, tag="xt", name="xt")
                    nc.sync.dma_start(xt[:, 0:4, :], xT[nch, :, 0:4, :])
                    nc.sync.dma_start(xt[:, 4:8, :], xT[nch, :, 4:8, :])
                c0 = nch * 512
                for w_sb, b_sb, dest in (
                    (wq_sb, bq_sb, qt_sb),
                    (wk_sb, bk_sb, kt_sb),
                    (wv_sb, bv_sb, None),
                ):
                    ps = ps_mm.tile([128, 512], F32, tag="mm", name="projps")
                    for kc in range(8):
                        nc.tensor.matmul(
                            ps[:],
                            w_sb[:, kc, :],
                            xt[:, kc, :],
                            start=(kc == 0),
                            stop=(kc == 7),
                        )
                    if dest is not None:
                        nc.vector.tensor_scalar_add(
                            dest[:, c0 : c0 + 512], ps[:], b_sb[:, 0:1]
                        )
                    else:
                        vtmp = vtmp_pool.tile([128, 512], BF16, tag="vtmp")
                        nc.vector.tensor_scalar(
                            vtmp[:], ps[:], b_sb[:, 0:1], 16.0,
                            mybir.AluOpType.add, mybir.AluOpType.mult,
                        )
                        if pending_tr[0] is not None:
                            emit_transposes(*pending_tr[0])
                        pending_tr[0] = (nch, vtmp)

            def proj_chunk_fillers(nch, xt_preset=None):
                state = {}
                if xt_preset is not None:
                    state["xt"] = xt_preset
                qa, qb = (nc.sync, nc.gpsimd) if nch % 2 else (nc.gpsimd, nc.sync)

                def load():
                    if "xt" in state:
                        return
                    xt = xt_pool.tile([128, 8, 512], BF16, tag="xt", name="xt")
                    qa.dma_start(xt[:, 0:4, :], xT[nch, :, 0:4, :])
                    qb.dma_start(xt[:, 4:8, :], xT[nch, :, 4:8, :])
                    state["xt"] = xt

                def group(w_sb, b_sb, dest):
                    # two 4-matmul halves so a filler burst never exceeds
                    # ~1us of PE time (keeps the exp train fed)
                    gstate = {}

                    def _a():
                        load()
                        xt = state["xt"]
                        ps = ps_mm.tile([128, 512], F32, tag="mm", name="projps")
                        gstate["ps"] = ps
                        for kc in range(4):
                            nc.tensor.matmul(
                                ps[:], w_sb[:, kc, :], xt[:, kc, :],
                                start=(kc == 0), stop=False,
                            )

                    def _b():
                        xt = state["xt"]
                        ps = gstate["ps"]
                        c0 = nch * 512
                        for kc in range(4, 8):
                            nc.tensor.matmul(
                                ps[:], w_sb[:, kc, :], xt[:, kc, :],
                                start=False, stop=(kc == 7),
                            )
                        if dest is not None:
                            nc.vector.tensor_scalar_add(
                                dest[:, c0 : c0 + 512], ps[:], b_sb[:, 0:1]
                            )
                        else:
                            vtmp = vtmp_pool.tile([128, 512], BF16, tag="vtmp")
                            nc.vector.tensor_scalar(
                                vtmp[:], ps[:], b_sb[:, 0:1], 16.0,
                                mybir.AluOpType.add, mybir.AluOpType.mult,
                            )
                            if pending_tr[0] is not None:
                                emit_transposes(*pending_tr[0])
                            pending_tr[0] = (nch, vtmp)
                    return _a, _b

                qa_, qb_ = group(wq_sb, bq_sb, qt_sb)
                ka_, kb_ = group(wk_sb, bk_sb, kt_sb)
                va_, vb_ = group(wv_sb, bv_sb, None)
                return {
                    "load": load,
                    "Qa": qa_, "Qb": qb_,
                    "Ka": ka_, "Kb": kb_,
                    "Va": va_, "Vb": vb_,
                    "Q": lambda: (qa_(), qb_()),
                    "K": lambda: (ka_(), kb_()),
                    "V": lambda: (va_(), vb_()),
                }

            def flush_tr():
                if pending_tr[0] is not None:
                    emit_transposes(*pending_tr[0])
                    pending_tr[0] = None

            # ---- phase B: attention units, software-pipelined ----
            # Per unit: 16-kt loop of (ST pair -> exp into fp8 pair tile ->
            # DoubleRow PV per kt-pair, lagged PV_LAG pairs). The previous
            # unit's finish (rowsum bcast + reciprocal + fused extract*norm)
            # runs at kt 1 of the current unit, and the unit-before-that's
            # output projection is spread over kt 8..15, so the PE never
            # drains at unit boundaries.
            PV_LAG = 2  # in kt-pairs

            # pre-zero the rowsum staging buffers (all 4 pool slots) so the
            # broadcast matmul's unused contraction rows 1..31 contribute
            # 0 * 0 instead of 0 * garbage
            for _ in range(4):
                _rs = small_pool.tile([33, 512], F32R, tag="rs")
                nc.vector.memset(_rs[:].bitcast(F32), 0.0)

            def emit_unit_kt(b, qc, slot_fns=None):
                slot_fns = dict(slot_fns or {})
                gq = b * s + qc * 512
                pvp0 = ps_pv.tile([65, 512], F32, tag="pv", name="pvp0")
                pvp1 = ps_pv.tile([65, 512], F32, tag="pv", name="pvp1")
                pv_tiles = [pvp0, pvp1]
                NP = KT // 2  # kt-pairs per unit

                def emit_pv(pair_v, pt_v):
                    if pair_v in SCHRAUD_PAIRS:
                        # bf16 PV off the high halves of the i32 Schraudolph
                        # words (bf16 = top 16 bits of f32)
                        ptv = pt_v[:].bitcast(BF16)  # [128, 2, 2048]
                        for h in range(2):
                            for g in range(2):
                                nc.tensor.matmul(
                                    pv_tiles[h][:],
                                    vp_bf[:, sch_tiles[b * KT + 2 * pair_v + g], h, 0:65],
                                    ptv[:, g, 1024 * h + 1 : 1024 * (h + 1) : 2],
                                    start=(pair_v == 0 and g == 0),
                                    stop=(pair_v == NP - 1 and g == 1),
                                )
                        return
                    # fp8 DoubleRow: one matmul covers both kt of the pair
                    # (contraction 256 keys); lhsT [128, 2, 65], rhs [128, 2, 512]
                    for h in range(2):
                        nc.tensor.matmul(
                            pv_tiles[h][:],
                            vp[:, b * KT + 2 * pair_v : b * KT + 2 * pair_v + 2, h, 0:65],
                            pt_v[:, :, h * 512 : h * 512 + 512],
                            start=(pair_v == 0),
                            stop=(pair_v == NP - 1),
                            perf_mode=mybir.MatmulPerfMode.DoubleRow,
                        )

                pending_pv = []
                pt_pair = None
                for kt in range(KT):
                    kcol = b * s + kt * 128
                    stp = ps_st.tile([128, 1024], F32, tag="st")
                    for h in range(2):
                        p0 = h * 64
                        nc.tensor.matmul(
                            stp[:, h * 512 : h * 512 + 512],
                            kt_sb[p0 : p0 + 64, kcol : kcol + 128],
                            qt_sb[p0 : p0 + 64, gq : gq + 512],
                            start=True,
                            stop=True,
                        )
                    sch = (kt // 2) in SCHRAUD_PAIRS
                    if kt % 2 == 0:
                        if sch:
                            pt_pair = pts_pool.tile(
                                [128, 2, 1024], I32, tag="pts", name="pts"
                            )
                        else:
                            pt_pair = pt_pool.tile(
                                [128, 2, 1024], F8, tag="pt", name="ptp"
                            )
                    if sch:
                        nc.vector.tensor_scalar(
                            pt_pair[:, kt % 2, :], stp[:], SCH_A, SCH_B,
                            mybir.AluOpType.mult, mybir.AluOpType.add,
                        )
                    else:
                        # pt = 8*exp(s): the 8 folds into the exp bias and
                        # cancels against the rowsum in normalization
                        nc.scalar.activation(
                            pt_pair[:, kt % 2, :], stp[:],
                            mybir.ActivationFunctionType.Exp, bias=ln8_sb[:, 0:1],
                        )
                    if kt % 2 == 1:
                        pending_pv.append((kt // 2, pt_pair))
                        if len(pending_pv) > PV_LAG - 1:
                            emit_pv(*pending_pv.pop(0))
                    for f in slot_fns.pop(kt, ()):
                        f()
                for kt in sorted(slot_fns):
                    for f in slot_fns.pop(kt):
                        f()
                # remaining PV pairs are flushed lazily by the caller (into
                # the next unit's first slots, after its first STs issue)
                return {"b": b, "qc": qc, "gq": gq, "pv": pv_tiles,
                        "emit_pv": emit_pv, "pending": pending_pv}

            def emit_finish_a(u):
                # rowsums (psum row 64 of each head) -> partitions 0/32 of one
                # sbuf tile -> single PE broadcast (contraction rows 1..31 are
                # zero in selc AND pre-zeroed in rs2) -> fast reciprocal
                rs2 = small_pool.tile([33, 512], F32R, tag="rs")
                nc.vector.tensor_copy(rs2[0:1, :], u["pv"][0][64:65, :])
                nc.vector.tensor_copy(rs2[32:33, :], u["pv"][1][64:65, :])
                # bcp lives in the mm pool so the ST psum ring stays a pure
                # (ST matmul -> exp) pipeline
                bcp = ps_mm.tile([128, 512], F32, tag="mm", name="bcp")
                nc.tensor.matmul(
                    bcp[:],
                    selc_sb[0:33, :],
                    rs2[0:33, :],
                    start=True,
                    stop=True,
                )
                bcs = bcs_pool.tile([128, 512], F32, tag="bcs")
                nc.vector.reciprocal_approx_fast(bcs[:], bcp[:])
                u["bcs"] = bcs

            def emit_finish_b(u):
                # extract attn bands from PSUM with the normalization fused in
                gq = u["gq"]
                bcs = u["bcs"]
                for h in range(2):
                    p0 = h * 64
                    nc.vector.tensor_tensor(
                        attn_sb[p0 : p0 + 64, gq : gq + 512],
                        u["pv"][h][0:64, :],
                        bcs[p0 : p0 + 64, :],
                        mybir.AluOpType.mult,
                    )

            def emit_unit_finish(u):
                emit_finish_a(u)
                emit_finish_b(u)

            def out_pieces(u, use_scalar=False, act_set=()):
                # 8 outproj pieces, each split into a matmul part and an
                # extract part so the extracts can land on the ACT inside
                # the Schraudolph window without their matmuls delaying STs
                gq = u["gq"]
                blk0 = gq // 128
                state = {}
                mms, exts = [], []
                for qb in range(4):
                    for no2 in range(2):
                        def _mm(qb=qb, no2=no2):
                            col = gq + qb * 128
                            ops = ps_mm.tile([128, 512], F32, tag="mm", name="ops")
                            state[(qb, no2)] = ops
                            nc.tensor.matmul(
                                ops[:],
                                attn_sb[:, col : col + 128],
                                wo_sb[:, no2 * 512 : (no2 + 1) * 512],
                                start=True,
                                stop=True,
                            )

                        def _ext(qb=qb, no2=no2):
                            qbp, qbs = qb // 2, qb % 2
                            if qbs == 0 and no2 == 0:
                                state[("osb", qbp)] = osb_pool.tile(
                                    [128, 2, E], BF16, tag="osb", name="osb"
                                )
                            osb = state[("osb", qbp)]
                            ops = state.pop((qb, no2))
                            dst = osb[:, qbs, no2 * 512 : (no2 + 1) * 512]
                            if (use_scalar and no2 == 1) or (qb * 2 + no2) in act_set:
                                nc.scalar.activation(
                                    dst, ops[:],
                                    mybir.ActivationFunctionType.Copy,
                                )
                            else:
                                nc.vector.tensor_copy(dst, ops[:])
                            if qbs == 1 and no2 == 1:
                                nc.sync.dma_start(
                                    outp[:, blk0 + 2 * qbp : blk0 + 2 * qbp + 2, :],
                                    osb[:, :, :],
                                )
                        mms.append(_mm)
                        exts.append(_ext)
                return mms, exts

            # Emission schedule (v2): minimal head -- chunk 0 (Q,K,V) and
            # chunk 1 (K,V) -- so the exp train starts as soon as chunk 0
            # lands. Everything else rides inside the units as deadline-
            # placed fillers; leftover PV pairs of each unit flush lazily at
            # the start of the next unit (after its first STs issue) so the
            # PE never serializes behind the ACT at unit boundaries.
            units = [(b_, qc_) for b_ in range(B) for qc_ in range(QC)]
            if NCH == 8:
                fill = {c: proj_chunk_fillers(c) for c in range(1, NCH)}
                fill[0] = proj_chunk_fillers(0, xt_preset=xt0)
                for c in (1, 2, 3):
                    fill[c]["load"]()
                fill[0]["Q"]()
                fill[0]["K"]()
                fill[0]["V"]()
                fill[1]["K"]()
                fill[1]["V"]()
                # unit-idx -> [(slot, kind, chunk)] with deadlines:
                #   K c by kt 4c of its batch-units; V c transposes by the
                #   PV pair that reads its vp tiles; Q c before unit c starts
                SLOTS = {
                    0: [(1, "Ka", 2), (2, "load", 4), (3, "Kb", 2),
                        (4, "Va", 2), (5, "Vb", 2), (6, "flush", 0),
                        (7, "Ka", 3), (8, "Va", 3), (9, "Kb", 3),
                        (10, "Vb", 3), (12, "flush", 0), (14, "Qa", 1),
                        (15, "Qb", 1)],
                    1: [(0, "load", 5), (3, "Ka", 4), (5, "Kb", 4),
                        (7, "Va", 4), (9, "Vb", 4), (14, "Qa", 2),
                        (15, "Qb", 2)],
                    2: [(0, "load", 6), (3, "Ka", 5), (5, "Kb", 5),
                        (7, "Va", 5), (9, "Vb", 5), (14, "Qa", 3),
                        (15, "Qb", 3)],
                    3: [(0, "load", 7), (3, "Ka", 6), (5, "Kb", 6),
                        (7, "Va", 6), (9, "Vb", 6), (14, "Qa", 4),
                        (15, "Qb", 4)],
                    4: [(3, "Ka", 7), (5, "Kb", 7), (7, "Va", 7),
                        (9, "Vb", 7), (11, "flush", 0), (14, "Qa", 5),
                        (15, "Qb", 5)],
                    5: [(14, "Qa", 6), (15, "Qb", 6)],
                    6: [(14, "Qa", 7), (15, "Qb", 7)],
                    7: [],
                }
            else:
                for nch in range(NCH):
                    emit_proj_chunk(nch)
                flush_tr()
                SLOTS = {i: [] for i in range(len(units))}
            prev = None
            carry_pv = None
            for idx, (b_, qc_) in enumerate(units):
                slots = {}
                if carry_pv is not None:
                    fn, pairs = carry_pv
                    for i, args in enumerate(pairs):
                        slots.setdefault(2 * i, []).append(
                            lambda a=args, f=fn: f(*a)
                        )
                if prev is not None:
                    # both finish halves must be emitted before this unit's
                    # first PV pop (kt3): the pv psum ring WAR only covers
                    # readers emitted before the ring's next first-write
                    slots.setdefault(1, []).append(
                        lambda u=prev: emit_finish_a(u)
                    )
                    slots.setdefault(2, []).append(
                        lambda u=prev: emit_finish_b(u)
                    )
                    mms, exts = out_pieces(prev, act_set=(0, 1))
                    for i in range(8):
                        sl = 4 + i if i < 2 else 5 + i
                        slots.setdefault(sl, []).append(mms[i])
                        slots.setdefault(sl, []).append(exts[i])
                for slot, kind, arg in SLOTS.get(idx, ()):
                    if kind == "load":
                        slots.setdefault(slot, []).append(fill[arg]["load"])
                    elif kind == "flush":
                        slots.setdefault(slot, []).append(flush_tr)
                    else:
                        slots.setdefault(slot, []).append(fill[arg][kind])
                u = emit_unit_kt(b_, qc_, slots)
                prev = u
                carry_pv = (u["emit_pv"], u["pending"])
            fn, pairs = carry_pv
            for args in pairs:
                fn(*args)
            emit_finish_a(prev)
            emit_finish_b(prev)
            mms, exts = out_pieces(prev, use_scalar=True)
            for m, e in zip(mms, exts):
                m()
                e()
            if debug:
                nc.sync.dma_start(dbg["dbg_qt"][:], qt_sb[:])
                nc.sync.dma_start(dbg["dbg_kt"][:], kt_sb[:])
                nc.sync.dma_start(dbg["dbg_vp0"][:], vp[:])
                nc.sync.dma_start(dbg["dbg_attn"][:], attn_sb[:])
    return nc


def _prep_inputs(inputs, Wq, bq, Wk, bk, Wv, bv, Wo, bo, s=S):
    """Host-side shard + relayout. Returns per-core input maps."""
    inputs = np.asarray(inputs, dtype=np.float32)
    Wq, Wk, Wv, Wo = (np.asarray(w, dtype=np.float32) for w in (Wq, Wk, Wv, Wo))
    bq, bk, bv = (np.asarray(b_, dtype=np.float32) for b_ in (bq, bk, bv))
    r = B * s
    x = np.ascontiguousarray(inputs, dtype=np.float32).reshape(r, E)
    # [E, r] -> per-512-chunk contiguous tiles [NCH, 128(part), 8(kc), 512]
    xT = np.ascontiguousarray(
        x.T.reshape(8, 128, r // 512, 512).transpose(2, 1, 0, 3)
    ).astype(ml_dtypes.bfloat16)

    # 16.0: folds the 1/16 v-prescale into the reciprocal of the rowsum
    selc = np.zeros((128, 128), dtype=np.float32)
    selc[0, 0:64] = 16.0
    selc[32, 64:128] = 16.0

    def wslice(W, c):
        # W[c*128:(c+1)*128, :] transposed -> [E, 128] -> [128(part), 8, 128]
        wt = np.ascontiguousarray(
            W[c * 128 : (c + 1) * 128, :].T.reshape(8, 128, 128).transpose(1, 0, 2)
        )
        return wt

    in_maps = []
    for c in range(NCORES):
        m = {
            "xT": xT,
            "wq": (wslice(Wq, c) * 0.125).astype(ml_dtypes.bfloat16),
            "wk": wslice(Wk, c).astype(ml_dtypes.bfloat16),
            "wv": wslice(Wv, c).astype(ml_dtypes.bfloat16),
            "bq": (bq[c * 128 : (c + 1) * 128] * 0.125).reshape(128, 1).astype(np.float32),
            "bk": bk[c * 128 : (c + 1) * 128].reshape(128, 1).astype(np.float32),
            "bv": bv[c * 128 : (c + 1) * 128].reshape(128, 1).astype(np.float32),
            "wo": np.ascontiguousarray(Wo[:, c * 128 : (c + 1) * 128].T).astype(
                ml_dtypes.bfloat16
            ),
            "selc": selc,
        }
        in_maps.append(m)
    return in_maps


def _get_nc(s=S):
    if s not in _CACHED:
        import concourse.mybir as mybir

        nc = _split_waits(build_nc(s))
        # populate .instr bytes for extended InstISA subclasses (the custom
        # DVE reciprocal) -- raw Bass skips this pass and the NEFF compiler
        # errors with "ISA wrong length" without it
        mybir.codegen_inst_isa_subclasses(nc)
        _CACHED[s] = nc
    return _CACHED[s]


def kernel(
    inputs, Wq, bq, Wk, bk, Wv, bv, Wo, bo, _trace=False, _result_box=None
):
    from concourse.bass_utils import run_bass_kernel_spmd

    nc = _get_nc(S)
    in_maps = _prep_inputs(inputs, Wq, bq, Wk, bk, Wv, bv, Wo, bo)
    res = run_bass_kernel_spmd(nc, in_maps, list(range(NCORES)), trace=_trace)
    if _result_box is not None:
        _result_box.append(res)
    acc = np.zeros((B * S, E), dtype=np.float32)
    for rmap in res.results:
        # device layout [128, r//128, E]: element [p, blk, c] = row blk*128+p
        part = np.asarray(rmap["out"], dtype=np.float32)
        acc += part.transpose(1, 0, 2).reshape(B * S, E)
    acc += np.asarray(bo, dtype=np.float32)[None, :]
    return acc.reshape(B, S, E)

